# revision 16
# baseline (speedup 1.0000x reference)
"""Trainium2 Bass kernel for nn_NegSimHead (loss_fn).

Reference computation (N=8192, C=512):
  v = normalize(v_feat); t = normalize(t_feat); pv = normalize(p_v); pt = normalize(p_t)
  neg_sim = -0.5*mean(sum(pv*t,1)) - 0.5*mean(sum(pt*v,1))
  stats(x) = mean(std(x, axis=0, ddof=1)) for each normalized tensor
  s1 = v @ pt.T ; s2 = t @ pv.T
  retrieval(s): pos[i] = rank of s[i,i] in row i (descending) = #{j: s[i,j] > s[i,i]}
  out [13] = [neg_sim, stats(v), stats(t), stats(pv), stats(pt),
              r1,r5,r10,mr of s1, r1,r5,r10,mr of s2]

Strategy (8 cores, data-parallel over N):
  Core k gets rows k*1024..(k+1)*1024 of v/t (transposed, [512,1024]) and the FULL
  p_t/p_v transposed and ROLLED by -k*1024 rows, so that local column j of the
  similarity strip corresponds to global row (k*1024+j)%N.  The diagonal of the
  full similarity matrix then sits at static local positions (strip mb's diagonal
  is in column block mb) identically on every core -> pure SPMD, no collectives.

  Row-normalization of v/t scales whole rows of s and cancels in the rank
  comparison, so raw vT/tT feed the matmul directly.  p̂T is column-normalized on
  device (squares -> all-ones matmul partition-reduce -> reciprocal -> sqrt ->
  scale).  Matmuls run in float32r (fp22 mantissa, full PE speed at free dim 512).
  The diagonal d is extracted bit-exactly from the matmul output (identity mask
  multiply + reduce), so the self-comparison contributes exactly zero.  Counting
  is split between ScalarE (Sign(s-d) with per-partition bias, fused accumulate)
  and VectorE (is_gt with per-partition scalar, fused accumulate).

  Per-core partials ([128,42]: per-feature stat sums/sumsqs, per-partition
  retrieval threshold counts + rank sums, loss sums) are AllReduce-summed
  across the 8 cores on device, and the final variance/sqrt/mean + metric
  division math also runs on device, so every core holds the identical final
  13-vector.  The host fetches 64B/core and slices.

Performance model (axon-tunneled trn2): every RPC round trip through the
tunnel costs ~80ms flat, so the measured "HW exec time" = one dispatch+fetch
round (~80ms) + NEFF launch (~2-5ms) + device exec (~2ms).  The optimization
history: 4 separate output fetches (~350ms) -> one batched fetch of [128,112]
partials (~95ms) -> on-device AllReduce + finish with a [1,16] output (~86ms).
"""
import time
import numpy as np
from contextlib import ExitStack

import concourse.bacc as bacc
import concourse.tile as tile
from concourse import mybir

F32 = mybir.dt.float32
F32R = mybir.dt.float32r
ALU = mybir.AluOpType
AX = mybir.AxisListType
AF = mybir.ActivationFunctionType

N = 8192          # batch
C = 512           # feature dim
NCORES = 8
S = N // NCORES   # rows per core = 1024
KC = C // 128     # contraction chunks = 4
MB = S // 128     # row strips per core = 8
NTILE = 512       # similarity column tile
NT = N // NTILE   # column tiles = 16
# column tiles counted on ScalarE (Sign) vs VectorE (is_gt).  Diagonal tiles
# (nt 0,1) must be on the DVE/is_gt path so exact ties (the diagonal) count 0.
ACT_SET = frozenset(range(2, 10))
C_ACT = len(ACT_SET) * NTILE

_CACHE = {}
RESULTS = None  # last BassKernelResults (exec_time_ns etc.), for test harnesses


def _build_program():
    nc = bacc.Bacc("TRN2", target_bir_lowering=False, debug=False,
                   num_devices=NCORES)

    xT_d = [nc.dram_tensor("vT", [C, S], F32R, kind="ExternalInput").ap(),
            nc.dram_tensor("tT", [C, S], F32R, kind="ExternalInput").ap()]
    P_d = [nc.dram_tensor("ptT", [C, N], F32R, kind="ExternalInput").ap(),
           nc.dram_tensor("pvT", [C, N], F32R, kind="ExternalInput").ap()]
    ident_d = nc.dram_tensor("ident", [128, 128], F32, kind="ExternalInput").ap()
    ones_d = nc.dram_tensor("ones", [128, 128], F32R, kind="ExternalInput").ap()

    # final 13-vector (padded to 16), identical on every core: the [128,42]
    # per-core partials are AllReduce-summed across the 8 cores on device,
    # then the variance/sqrt/mean + metric division math runs on device too.
    # Output payload is 64B/core => the d2h fetch round trip (~81ms on axon)
    # carries essentially nothing beyond its fixed latency.
    o_all_d = nc.dram_tensor("o_all", [1, 16], F32, kind="ExternalOutput").ap()

    with tile.TileContext(nc) as tc, ExitStack() as ctx:
        persist = ctx.enter_context(tc.tile_pool(name="persist", bufs=1))
        ppool = ctx.enter_context(tc.tile_pool(name="ppool", bufs=1))
        sq_pool = ctx.enter_context(tc.tile_pool(name="sq", bufs=2))
        b_pool = ctx.enter_context(tc.tile_pool(name="bb", bufs=2))
        xh_pool = ctx.enter_context(tc.tile_pool(name="xh", bufs=2))
        scr_pool = ctx.enter_context(tc.tile_pool(name="scr", bufs=2))
        mm_psum = ctx.enter_context(tc.tile_pool(name="mmps", bufs=7, space="PSUM"))
        nrm_psum = ctx.enter_context(tc.tile_pool(name="nrmps", bufs=1, space="PSUM"))
        dram_pool = ctx.enter_context(tc.tile_pool(name="dram", bufs=2,
                                                   space="DRAM"))

        i_t = persist.tile([128, 128], F32, name="i_t")
        ones_t = persist.tile([128, 128], F32R, name="ones_t")
        nc.sync.dma_start(out=i_t, in_=ident_d)
        nc.sync.dma_start(out=ones_t, in_=ones_d)

        o_sgn = persist.tile([128, 2 * MB], F32, name="o_sgn")
        o_cnt = persist.tile([128, 2 * MB], F32, name="o_cnt")
        o_stats = persist.tile([128, 64], F32, name="o_stats")
        o_loss = persist.tile([128, 16], F32, name="o_loss")

        # x shards stay resident for the whole kernel
        xT = [[persist.tile([128, S], F32R, name=f"xT{ph}_{k}")
               for k in range(KC)] for ph in range(2)]

        # persistent per-phase state
        P = [[[None] * NT for _ in range(KC)] for _ in range(2)]
        invb_x = [persist.tile([128, S], F32, name=f"invb_x{ph2}")
                  for ph2 in range(2)]
        d_sb = [persist.tile([128, MB], F32, name=f"d{ph2}") for ph2 in range(2)]
        negd_sb = [persist.tile([128, MB], F32, name=f"negd{ph2}")
                   for ph2 in range(2)]
        cnts = [persist.tile([128, MB, NT], F32, name=f"cnts{ph2}")
                for ph2 in range(2)]
        sgns = [persist.tile([128, MB, NT], F32, name=f"sgns{ph2}")
                for ph2 in range(2)]
        for ph in range(2):
            nc.vector.memset(cnts[ph], 0.0)
            nc.vector.memset(sgns[ph], 0.0)

        def x_norm(ph):
            for h in range(2):
                hs = slice(h * 512, (h + 1) * 512)
                ps_x = nrm_psum.tile([128, 512], F32, name=f"psx{ph}_{h}",
                                     tag="nrm")
                for k in range(KC):
                    sqx = sq_pool.tile([128, 512], F32R,
                                       name=f"sqx{ph}_{k}_{h}", tag="sqx")
                    nc.scalar.square(sqx, xT[ph][k][:, hs])
                    nc.tensor.matmul(ps_x, ones_t, sqx,
                                     start=(k == 0), stop=(k == KC - 1))
                nc.vector.reciprocal(invb_x[ph][:, hs], ps_x)
                nc.scalar.sqrt(invb_x[ph][:, hs], invb_x[ph][:, hs])

        def load_and_norm_P(ph, nt):
            # DMA the 4 contraction chunks of column tile nt, then column-
            # normalize in place (squares -> all-ones matmul partition-sum ->
            # rsqrt -> scale).  Tags are shared across phases (bufs=1), so
            # phase 1's DMA naturally waits for phase 0's last reader.
            for k in range(KC):
                pt_ = ppool.tile([128, NTILE], F32R, name=f"P_{k}_{nt}",
                                 tag=f"P_{k}_{nt}")
                nc.sync.dma_start(
                    out=pt_, in_=P_d[ph][k * 128:(k + 1) * 128,
                                         nt * NTILE:(nt + 1) * NTILE])
                P[ph][k][nt] = pt_
            ps_n = nrm_psum.tile([128, NTILE], F32, name=f"psn{ph}_{nt}",
                                 tag="nrm")
            for k in range(KC):
                sq = sq_pool.tile([128, NTILE], F32R, name=f"sq{ph}_{nt}_{k}",
                                  tag="sq")
                nc.scalar.square(sq, P[ph][k][nt])
                nc.tensor.matmul(ps_n, ones_t, sq,
                                 start=(k == 0), stop=(k == KC - 1))
            b_t = b_pool.tile([128, NTILE], F32, name=f"b{ph}_{nt}", tag="b")
            nc.vector.reciprocal(b_t, ps_n)
            nc.scalar.sqrt(b_t, b_t)
            for k in range(KC):
                nc.vector.tensor_mul(P[ph][k][nt], P[ph][k][nt], b_t)

        def stats_chunk(ph, k):
            # stats tensor index: x side: v(0), t(1); P side: pt(3), pv(2)
            # column layout: h*32 + {0:sum, 16:sumsq} + pstat*4 + k, so the
            # two h-halves fold with one [128,32] tensor_add and the folded
            # block is [s(16) | ssq(16)] for vectorized variance math.
            pstat = 3 if ph == 0 else 2
            for h in range(2):
                col = h * 32 + pstat * 4 + k
                nc.vector.tensor_reduce(o_stats[:, col:col + 1],
                                        P[ph][k][h], axis=AX.X, op=ALU.add)
                pscr = scr_pool.tile([128, NTILE], F32,
                                     name=f"pscr{ph}_{k}_{h}", tag="scr")
                nc.scalar.activation(out=pscr, in_=P[ph][k][h],
                                     func=AF.Square,
                                     accum_out=o_stats[:, col + 16:col + 17])
            xh = xh_pool.tile([128, S], F32, name=f"xh{ph}_{k}", tag="xh")
            nc.vector.tensor_mul(xh, xT[ph][k], invb_x[ph])
            for h in range(2):
                col = h * 32 + ph * 4 + k
                hs = slice(h * 512, (h + 1) * 512)
                nc.vector.tensor_reduce(o_stats[:, col:col + 1], xh[:, hs],
                                        axis=AX.X, op=ALU.add)
                xscr = scr_pool.tile([128, 512], F32,
                                     name=f"xscr{ph}_{k}_{h}", tag="scr")
                nc.scalar.activation(out=xscr, in_=xh[:, hs],
                                     func=AF.Square,
                                     accum_out=o_stats[:, col + 16:col + 17])
                # loss: sum(x-hat * p-hat) over own shard rows
                lscr = scr_pool.tile([128, 512], F32,
                                     name=f"lscr{ph}_{k}_{h}", tag="scr")
                nc.vector.tensor_mul(lscr, xh[:, hs], P[ph][k][h])
                lcol = ph * 8 + k * 2 + h
                nc.vector.tensor_reduce(o_loss[:, lcol:lcol + 1], lscr,
                                        axis=AX.X, op=ALU.add)

        def mm_strip(ph, mb, nt):
            ps = mm_psum.tile([128, NTILE], F32, name=f"ps{ph}_{mb}_{nt}",
                              tag="mm")
            for k in range(KC):
                nc.tensor.matmul(ps, xT[ph][k][:, mb * 128:(mb + 1) * 128],
                                 P[ph][k][nt], start=(k == 0),
                                 stop=(k == KC - 1))
            return ps

        def d_pass(ph):
            # for each strip, compute its diagonal-containing tile first,
            # extract d (bit-exact: identity-mask multiply + reduce), and
            # count that tile on the DVE/is_gt path (self-comparison = 0)
            for mb in range(MB):
                nt_d = (mb * 128) // NTILE
                ps = mm_strip(ph, mb, nt_d)
                sub = (mb * 128) % NTILE
                dscr = scr_pool.tile([128, 128], F32, name=f"dscr{ph}_{mb}",
                                     tag="dscr")
                nc.vector.tensor_mul(dscr, ps[:, sub:sub + 128], i_t)
                nc.vector.tensor_reduce(d_sb[ph][:, mb:mb + 1], dscr,
                                        axis=AX.X, op=ALU.add)
                nc.vector.tensor_scalar_mul(negd_sb[ph][:, mb:mb + 1],
                                            d_sb[ph][:, mb:mb + 1], -1.0)
                cscr = scr_pool.tile([128, NTILE], F32, name=f"cscr{ph}_{mb}",
                                     tag="cscr")
                nc.vector.tensor_scalar(
                    out=cscr, in0=ps, scalar1=d_sb[ph][:, mb:mb + 1],
                    scalar2=0.0, op0=ALU.is_gt, op1=ALU.add,
                    accum_out=cnts[ph][:, mb, nt_d:nt_d + 1])

        def main_col(ph, nt):
            for mb in range(MB):
                if nt == (mb * 128) // NTILE:
                    continue  # handled in the d-pass
                ps = mm_strip(ph, mb, nt)
                if nt in ACT_SET:
                    ascr = scr_pool.tile([128, NTILE], F32,
                                         name=f"a{ph}_{nt}_{mb}", tag="ascr")
                    nc.scalar.activation(
                        out=ascr, in_=ps, func=AF.Sign,
                        bias=negd_sb[ph][:, mb:mb + 1], scale=1.0,
                        accum_out=sgns[ph][:, mb, nt:nt + 1])
                else:
                    cscr = scr_pool.tile([128, NTILE], F32,
                                         name=f"c{ph}_{nt}_{mb}", tag="cscr")
                    nc.vector.tensor_scalar(
                        out=cscr, in0=ps, scalar1=d_sb[ph][:, mb:mb + 1],
                        scalar2=0.0, op0=ALU.is_gt, op1=ALU.add,
                        accum_out=cnts[ph][:, mb, nt:nt + 1])

        def reduce_slots(ph):
            for mb in range(MB):
                c = ph * MB + mb
                nc.vector.tensor_reduce(o_cnt[:, c:c + 1], cnts[ph][:, mb, :],
                                        axis=AX.X, op=ALU.add)
                nc.vector.tensor_reduce(o_sgn[:, c:c + 1], sgns[ph][:, mb, :],
                                        axis=AX.X, op=ALU.add)

        def finalize():
            # per-core partials, packed: [0:16]=stat sums, [16:32]=stat
            # sumsqs (pstat*4+k), [32:40]=retrieval (ph*4+{r1,r5,r10,psum}),
            # [40:42]=loss (ph)
            o_cc = persist.tile([128, 42], F32, name="o_cc")
            # fold the two h-halves of the stats partials: [128,64] -> [128,32]
            nc.vector.tensor_add(o_cc[:, 0:32], o_stats[:, 0:32],
                                 o_stats[:, 32:64])
            # retrieval: pos = cnt + (sgn + C_ACT)/2 per row, then threshold
            # counts + pos-sum per partition
            for ph in range(2):
                sl = slice(ph * MB, (ph + 1) * MB)
                pos = scr_pool.tile([128, MB], F32, name=f"pos{ph}", tag="pos")
                nc.vector.tensor_scalar(out=pos, in0=o_sgn[:, sl],
                                        scalar1=float(C_ACT), scalar2=0.5,
                                        op0=ALU.add, op1=ALU.mult)
                nc.vector.tensor_add(pos, pos, o_cnt[:, sl])
                for j, thr in enumerate((1.0, 5.0, 10.0)):
                    tscr = scr_pool.tile([128, MB], F32,
                                         name=f"tscr{ph}_{j}", tag="tscr")
                    c = 32 + ph * 4 + j
                    nc.vector.tensor_scalar(
                        out=tscr, in0=pos, scalar1=thr, scalar2=0.0,
                        op0=ALU.is_lt, op1=ALU.add,
                        accum_out=o_cc[:, c:c + 1])
                nc.vector.tensor_reduce(o_cc[:, 35 + ph * 4:36 + ph * 4], pos,
                                        axis=AX.X, op=ALU.add)
                nc.vector.tensor_reduce(o_cc[:, 40 + ph:41 + ph],
                                        o_loss[:, ph * 8:(ph + 1) * 8],
                                        axis=AX.X, op=ALU.add)

            # ---- cross-core AllReduce of the partials (16.8KB, on-device) ----
            cc_in = dram_pool.tile([128, 42], F32, name="cc_in")
            cc_out = dram_pool.tile([128, 42], F32, name="cc_out")
            nc.gpsimd.dma_start(cc_in, o_cc)
            nc.gpsimd.collective_compute(
                "AllReduce", ALU.add,
                replica_groups=[list(range(NCORES))],
                ins=[cc_in.opt()], outs=[cc_out.opt()])
            g = persist.tile([128, 42], F32, name="g")
            nc.gpsimd.dma_start(g, cc_out)

            # ---- final math on device (identical on every core) ----
            # std per feature: var = (ssq - s*s/N)/(N-1), clamped, sqrt
            h_t = persist.tile([128, 26], F32, name="h_t")
            var = scr_pool.tile([128, 16], F32, name="var", tag="var")
            nc.vector.tensor_mul(var, g[:, 0:16], g[:, 0:16])
            nc.vector.tensor_scalar_mul(var, var, -1.0 / N)
            nc.vector.tensor_add(var, var, g[:, 16:32])
            nc.vector.tensor_scalar_mul(var, var, 1.0 / (N - 1))
            nc.vector.tensor_scalar_max(var, var, 0.0)
            nc.scalar.sqrt(h_t[:, 0:16], var)
            nc.vector.tensor_copy(h_t[:, 16:26], g[:, 32:42])
            # cross-partition sums via ones-vector matmul: [1,26]
            ones1 = persist.tile([128, 1], F32, name="ones1")
            nc.vector.memset(ones1, 1.0)
            ps_t = nrm_psum.tile([128, 26], F32, name="ps_fin", tag="nrm")
            ps_f = ps_t[0:1, :]
            nc.tensor.matmul(ps_f, ones1, h_t, start=True, stop=True)
            # assemble the 13-vector: [neg_sim, stats v,t,pv,pt, v_r1,v_r5,
            # v_r10,v_mr, t_r1,t_r5,t_r10,t_mr, 0,0,0]
            o_fin = persist.tile([1, 16], F32, name="o_fin")
            nc.vector.memset(o_fin, 0.0)
            for t in range(4):  # mean std over the 512 feature columns
                nc.vector.tensor_reduce(o_fin[:, 1 + t:2 + t],
                                        ps_f[:, t * 4:(t + 1) * 4],
                                        axis=AX.X, op=ALU.add)
            nc.vector.tensor_scalar_mul(o_fin[:, 1:5], o_fin[:, 1:5], 1.0 / C)
            nc.vector.tensor_scalar_mul(o_fin[:, 5:13], ps_f[:, 16:24], 1.0 / N)
            lsum = scr_pool.tile([1, 1], F32, name="lsum", tag="lsum")
            nc.vector.tensor_reduce(lsum, ps_f[:, 24:26], axis=AX.X, op=ALU.add)
            nc.vector.tensor_scalar_mul(o_fin[:, 0:1], lsum, -0.5 / N)
            nc.sync.dma_start(out=o_all_d, in_=o_fin)

        # ---- emission order (Tile priority / engine-FIFO order follows
        # program order, so interleave cross-phase work deliberately):
        # the P-column load+normalize stream leads the matmul+count stream by
        # two columns, and phase 1's loads trail phase 0's last reader. ----
        load_and_norm_P(0, 0)
        for k in range(KC):
            nc.sync.dma_start(out=xT[0][k],
                              in_=xT_d[0][k * 128:(k + 1) * 128, :])
        load_and_norm_P(0, 1)
        x_norm(0)
        d_pass(0)
        for nt in range(2, NT):
            load_and_norm_P(0, nt)
            m = nt - 2
            main_col(0, m)
            if m < KC:
                stats_chunk(0, m)
            if m == KC:
                for k in range(KC):
                    nc.sync.dma_start(out=xT[1][k],
                                      in_=xT_d[1][k * 128:(k + 1) * 128, :])
                x_norm(1)
            if m >= 5:
                load_and_norm_P(1, m - 5)
        main_col(0, NT - 2)
        load_and_norm_P(1, 9)
        main_col(0, NT - 1)
        load_and_norm_P(1, 10)
        for j in range(11, NT):
            load_and_norm_P(1, j)
        reduce_slots(0)
        d_pass(1)
        for nt in range(NT):
            main_col(1, nt)
            if nt < KC:
                stats_chunk(1, nt)
        reduce_slots(1)
        finalize()

    nc.compile()
    return nc


def _get_runner():
    """Build (once) a jitted 8-core SPMD executor for the Bass program.

    Mirrors bass2jax.run_bass_via_pjrt's multi-core branch, but keeps the
    jitted function and pre-staged device inputs so repeated calls skip
    retracing/recompiling, and so transfer vs execute can be timed apart.
    """
    if "runner" in _CACHE:
        return _CACHE["runner"]

    import jax
    import jax.numpy as jnp
    from jax.experimental.shard_map import shard_map
    from jax.sharding import Mesh, PartitionSpec, NamedSharding
    from concourse import mybir as _mybir
    from concourse.bass2jax import (_bass_exec_p, install_neuronx_cc_hook,
                                    partition_id_tensor)

    nc = _CACHE["nc"]
    install_neuronx_cc_hook()

    partition_name = (nc.partition_id_tensor.name
                      if nc.partition_id_tensor else None)
    in_names, out_names, out_avals = [], [], []
    zero_outs = []
    for alloc in nc.m.functions[0].allocations:
        if not isinstance(alloc, _mybir.MemoryLocationSet):
            continue
        name = alloc.memorylocations[0].name
        if alloc.kind == "ExternalInput":
            if name != partition_name:
                in_names.append(name)
        elif alloc.kind == "ExternalOutput":
            out_names.append(name)
            shape = tuple(alloc.tensor_shape)
            dtype = _mybir.dt.np(alloc.dtype)
            out_avals.append(jax.core.ShapedArray(shape, dtype))
            zero_outs.append(np.zeros(shape, dtype))
    n_params = len(in_names)
    all_in_names = in_names + out_names
    if partition_name is not None:
        all_in_names = all_in_names + [partition_name]

    def _body(*args):
        operands = list(args)
        if partition_name is not None:
            operands.append(partition_id_tensor())
        outs = _bass_exec_p.bind(
            *operands,
            out_avals=tuple(out_avals),
            in_names=tuple(all_in_names),
            out_names=tuple(out_names),
            lowering_input_output_aliases=(),
            sim_require_finite=True,
            sim_require_nnan=True,
            nc=nc,
        )
        return tuple(outs)

    devices = jax.devices()[:NCORES]
    mesh = Mesh(np.asarray(devices), ("core",))
    spec = NamedSharding(mesh, PartitionSpec("core"))
    donate = tuple(range(n_params, n_params + len(out_names)))
    sharded = jax.jit(
        shard_map(_body, mesh=mesh,
                  in_specs=(PartitionSpec("core"),) * (n_params + len(out_names)),
                  out_specs=(PartitionSpec("core"),) * len(out_names),
                  check_rep=False),
        donate_argnums=donate, keep_unused=True)

    def run(in_maps, _retries=2):
        t0 = time.time()
        concat_in = [
            np.concatenate([in_maps[c][name] for c in range(NCORES)], axis=0)
            for name in in_names
        ]
        try:
            return _run_staged(concat_in, t0)
        except Exception:
            # devices occasionally wedge (NRT_EXEC_UNIT_UNRECOVERABLE);
            # a fresh attempt sometimes recovers within the process
            if _retries <= 0:
                raise
            time.sleep(2.0)
            return run(in_maps, _retries - 1)

    def _run_staged(concat_in, t0):
        dev_in = [jax.device_put(a, spec) for a in concat_in]
        dev_zero = [jax.device_put(
            np.zeros((NCORES * z.shape[0], *z.shape[1:]), z.dtype), spec)
            for z in zero_outs]
        for a in dev_in + dev_zero:
            a.block_until_ready()
        t1 = time.time()
        out_arrs = sharded(*dev_in, *dev_zero)
        td = time.time()
        if FETCH_SHARD0:
            # every core returns the identical reduced vector, so fetch only
            # device 0's shard (one d2h RPC instead of eight overlapped ones)
            shard0 = [np.asarray(a.addressable_shards[0].data)
                      for a in out_arrs]
            t2 = time.time()
            TIMES.update(transfer_s=t1 - t0, execute_s=t2 - t1,
                         dispatch_s=td - t1, fetch_s=t2 - td)
            return [
                {name: shard0[i] for i, name in enumerate(out_names)}
                for c in range(NCORES)
            ]
        # one batched fetch: d2h rounds through the axon tunnel cost ~81ms
        # each but overlap perfectly, so fetch everything in one call
        out_np = jax.device_get(list(out_arrs))
        t2 = time.time()
        TIMES.update(transfer_s=t1 - t0, execute_s=t2 - t1,
                     dispatch_s=td - t1, fetch_s=t2 - td)
        return [
            {name: out_np[i].reshape(NCORES, *out_avals[i].shape)[c]
             for i, name in enumerate(out_names)}
            for c in range(NCORES)
        ]

    _CACHE["runner"] = run
    return run


TIMES = {}
FETCH_SHARD0 = False


def kernel(v_feat, t_feat, p_v, p_t):
    if "nc" not in _CACHE:
        _CACHE["nc"] = _build_program()

    t0 = time.time()
    v = np.ascontiguousarray(v_feat, dtype=np.float32)
    t = np.ascontiguousarray(t_feat, dtype=np.float32)
    pv = np.ascontiguousarray(p_v, dtype=np.float32)
    pt = np.ascontiguousarray(p_t, dtype=np.float32)

    ident = np.eye(128, dtype=np.float32)
    ones = np.ones((128, 128), dtype=np.float32)

    in_maps = []
    for k in range(NCORES):
        sl = slice(k * S, (k + 1) * S)
        in_maps.append({
            "vT": np.ascontiguousarray(v[sl].T),
            "tT": np.ascontiguousarray(t[sl].T),
            "ptT": np.ascontiguousarray(np.roll(pt, -k * S, axis=0).T),
            "pvT": np.ascontiguousarray(np.roll(pv, -k * S, axis=0).T),
            "ident": ident,
            "ones": ones,
        })
    TIMES["prep_s"] = time.time() - t0

    results = _get_runner()(in_maps)

    # the whole reduction ran on device (AllReduce across cores + final
    # math); every core returns the identical padded 13-vector
    out = np.asarray(results[0]["o_all"][0][:13], dtype=np.float32)
    return out



# revision 17
# speedup vs baseline: 1.1040x; 1.1040x over previous
"""Trainium2 Bass kernel for nn_NegSimHead (loss_fn).

Reference computation (N=8192, C=512):
  v = normalize(v_feat); t = normalize(t_feat); pv = normalize(p_v); pt = normalize(p_t)
  neg_sim = -0.5*mean(sum(pv*t,1)) - 0.5*mean(sum(pt*v,1))
  stats(x) = mean(std(x, axis=0, ddof=1)) for each normalized tensor
  s1 = v @ pt.T ; s2 = t @ pv.T
  retrieval(s): pos[i] = rank of s[i,i] in row i (descending) = #{j: s[i,j] > s[i,i]}
  out [13] = [neg_sim, stats(v), stats(t), stats(pv), stats(pt),
              r1,r5,r10,mr of s1, r1,r5,r10,mr of s2]

Strategy (8 cores, data-parallel over N):
  Core k gets rows k*1024..(k+1)*1024 of v/t (transposed, [512,1024]) and the FULL
  p_t/p_v transposed and ROLLED by -k*1024 rows, so that local column j of the
  similarity strip corresponds to global row (k*1024+j)%N.  The diagonal of the
  full similarity matrix then sits at static local positions (strip mb's diagonal
  is in column block mb) identically on every core -> pure SPMD, no collectives.

  Row-normalization of v/t scales whole rows of s and cancels in the rank
  comparison, so raw vT/tT feed the matmul directly.  p̂T is column-normalized on
  device (squares -> all-ones matmul partition-reduce -> reciprocal -> sqrt ->
  scale).  Matmuls run in float32r (fp22 mantissa, full PE speed at free dim 512).
  The diagonal d is extracted bit-exactly from the matmul output (identity mask
  multiply + reduce), so the self-comparison contributes exactly zero.  Counting
  is split between ScalarE (Sign(s-d) with per-partition bias, fused accumulate)
  and VectorE (is_gt with per-partition scalar, fused accumulate).

  Per-core partials ([128,42]: per-feature stat sums/sumsqs, per-partition
  retrieval threshold counts + rank sums, loss sums) are AllReduce-summed
  across the 8 cores on device, and the final variance/sqrt/mean + metric
  division math also runs on device, so every core holds the identical final
  13-vector.  The host fetches 64B/core and slices.

Performance model (axon-tunneled trn2): every RPC round trip through the
tunnel costs ~80ms flat, so the measured "HW exec time" = one dispatch+fetch
round (~80ms) + NEFF launch (~2-5ms) + device exec (~2ms).  The optimization
history: 4 separate output fetches (~350ms) -> one batched fetch of [128,112]
partials (~95ms) -> on-device AllReduce + finish with a [1,16] output (~86ms).
"""
import time
import numpy as np
from contextlib import ExitStack

import concourse.bacc as bacc
import concourse.tile as tile
from concourse import mybir

F32 = mybir.dt.float32
F32R = mybir.dt.float32r
ALU = mybir.AluOpType
AX = mybir.AxisListType
AF = mybir.ActivationFunctionType

N = 8192          # batch
C = 512           # feature dim
NCORES = 8
S = N // NCORES   # rows per core = 1024
KC = C // 128     # contraction chunks = 4
MB = S // 128     # row strips per core = 8
NTILE = 512       # similarity column tile
NT = N // NTILE   # column tiles = 16
# column tiles counted on ScalarE (Sign) vs VectorE (is_gt).  Diagonal tiles
# (nt 0,1) must be on the DVE/is_gt path so exact ties (the diagonal) count 0.
ACT_SET = frozenset(range(2, 10))
C_ACT = len(ACT_SET) * NTILE

_CACHE = {}
RESULTS = None  # last BassKernelResults (exec_time_ns etc.), for test harnesses


def _build_program():
    nc = bacc.Bacc("TRN2", target_bir_lowering=False, debug=False,
                   num_devices=NCORES)

    xT_d = [nc.dram_tensor("vT", [C, S], F32R, kind="ExternalInput").ap(),
            nc.dram_tensor("tT", [C, S], F32R, kind="ExternalInput").ap()]
    P_d = [nc.dram_tensor("ptT", [C, N], F32R, kind="ExternalInput").ap(),
           nc.dram_tensor("pvT", [C, N], F32R, kind="ExternalInput").ap()]
    ident_d = nc.dram_tensor("ident", [128, 128], F32, kind="ExternalInput").ap()
    ones_d = nc.dram_tensor("ones", [128, 128], F32R, kind="ExternalInput").ap()

    # final 13-vector (padded to 16), identical on every core: the [128,42]
    # per-core partials are AllReduce-summed across the 8 cores on device,
    # then the variance/sqrt/mean + metric division math runs on device too.
    # Output payload is 64B/core => the d2h fetch round trip (~81ms on axon)
    # carries essentially nothing beyond its fixed latency.
    o_all_d = nc.dram_tensor("o_all", [1, 16], F32, kind="ExternalOutput").ap()

    with tile.TileContext(nc) as tc, ExitStack() as ctx:
        persist = ctx.enter_context(tc.tile_pool(name="persist", bufs=1))
        ppool = ctx.enter_context(tc.tile_pool(name="ppool", bufs=1))
        sq_pool = ctx.enter_context(tc.tile_pool(name="sq", bufs=2))
        b_pool = ctx.enter_context(tc.tile_pool(name="bb", bufs=2))
        xh_pool = ctx.enter_context(tc.tile_pool(name="xh", bufs=2))
        scr_pool = ctx.enter_context(tc.tile_pool(name="scr", bufs=2))
        mm_psum = ctx.enter_context(tc.tile_pool(name="mmps", bufs=7, space="PSUM"))
        nrm_psum = ctx.enter_context(tc.tile_pool(name="nrmps", bufs=1, space="PSUM"))
        dram_pool = ctx.enter_context(tc.tile_pool(name="dram", bufs=2,
                                                   space="DRAM"))

        i_t = persist.tile([128, 128], F32, name="i_t")
        ones_t = persist.tile([128, 128], F32R, name="ones_t")
        nc.sync.dma_start(out=i_t, in_=ident_d)
        nc.sync.dma_start(out=ones_t, in_=ones_d)

        o_sgn = persist.tile([128, 2 * MB], F32, name="o_sgn")
        o_cnt = persist.tile([128, 2 * MB], F32, name="o_cnt")
        o_stats = persist.tile([128, 64], F32, name="o_stats")
        o_loss = persist.tile([128, 16], F32, name="o_loss")

        # x shards stay resident for the whole kernel
        xT = [[persist.tile([128, S], F32R, name=f"xT{ph}_{k}")
               for k in range(KC)] for ph in range(2)]

        # persistent per-phase state
        P = [[[None] * NT for _ in range(KC)] for _ in range(2)]
        invb_x = [persist.tile([128, S], F32, name=f"invb_x{ph2}")
                  for ph2 in range(2)]
        d_sb = [persist.tile([128, MB], F32, name=f"d{ph2}") for ph2 in range(2)]
        negd_sb = [persist.tile([128, MB], F32, name=f"negd{ph2}")
                   for ph2 in range(2)]
        cnts = [persist.tile([128, MB, NT], F32, name=f"cnts{ph2}")
                for ph2 in range(2)]
        sgns = [persist.tile([128, MB, NT], F32, name=f"sgns{ph2}")
                for ph2 in range(2)]
        for ph in range(2):
            nc.vector.memset(cnts[ph], 0.0)
            nc.vector.memset(sgns[ph], 0.0)

        def x_norm(ph):
            for h in range(2):
                hs = slice(h * 512, (h + 1) * 512)
                ps_x = nrm_psum.tile([128, 512], F32, name=f"psx{ph}_{h}",
                                     tag="nrm")
                for k in range(KC):
                    sqx = sq_pool.tile([128, 512], F32R,
                                       name=f"sqx{ph}_{k}_{h}", tag="sqx")
                    nc.scalar.square(sqx, xT[ph][k][:, hs])
                    nc.tensor.matmul(ps_x, ones_t, sqx,
                                     start=(k == 0), stop=(k == KC - 1))
                nc.vector.reciprocal(invb_x[ph][:, hs], ps_x)
                nc.scalar.sqrt(invb_x[ph][:, hs], invb_x[ph][:, hs])

        def load_and_norm_P(ph, nt):
            # DMA the 4 contraction chunks of column tile nt, then column-
            # normalize in place (squares -> all-ones matmul partition-sum ->
            # rsqrt -> scale).  Tags are shared across phases (bufs=1), so
            # phase 1's DMA naturally waits for phase 0's last reader.
            for k in range(KC):
                pt_ = ppool.tile([128, NTILE], F32R, name=f"P_{k}_{nt}",
                                 tag=f"P_{k}_{nt}")
                nc.sync.dma_start(
                    out=pt_, in_=P_d[ph][k * 128:(k + 1) * 128,
                                         nt * NTILE:(nt + 1) * NTILE])
                P[ph][k][nt] = pt_
            ps_n = nrm_psum.tile([128, NTILE], F32, name=f"psn{ph}_{nt}",
                                 tag="nrm")
            for k in range(KC):
                sq = sq_pool.tile([128, NTILE], F32R, name=f"sq{ph}_{nt}_{k}",
                                  tag="sq")
                nc.scalar.square(sq, P[ph][k][nt])
                nc.tensor.matmul(ps_n, ones_t, sq,
                                 start=(k == 0), stop=(k == KC - 1))
            b_t = b_pool.tile([128, NTILE], F32, name=f"b{ph}_{nt}", tag="b")
            nc.vector.reciprocal(b_t, ps_n)
            nc.scalar.sqrt(b_t, b_t)
            for k in range(KC):
                nc.vector.tensor_mul(P[ph][k][nt], P[ph][k][nt], b_t)

        def stats_chunk(ph, k):
            # stats tensor index: x side: v(0), t(1); P side: pt(3), pv(2)
            # column layout: h*32 + {0:sum, 16:sumsq} + pstat*4 + k, so the
            # two h-halves fold with one [128,32] tensor_add and the folded
            # block is [s(16) | ssq(16)] for vectorized variance math.
            pstat = 3 if ph == 0 else 2
            for h in range(2):
                col = h * 32 + pstat * 4 + k
                nc.vector.tensor_reduce(o_stats[:, col:col + 1],
                                        P[ph][k][h], axis=AX.X, op=ALU.add)
                pscr = scr_pool.tile([128, NTILE], F32,
                                     name=f"pscr{ph}_{k}_{h}", tag="scr")
                nc.scalar.activation(out=pscr, in_=P[ph][k][h],
                                     func=AF.Square,
                                     accum_out=o_stats[:, col + 16:col + 17])
            xh = xh_pool.tile([128, S], F32, name=f"xh{ph}_{k}", tag="xh")
            nc.vector.tensor_mul(xh, xT[ph][k], invb_x[ph])
            for h in range(2):
                col = h * 32 + ph * 4 + k
                hs = slice(h * 512, (h + 1) * 512)
                nc.vector.tensor_reduce(o_stats[:, col:col + 1], xh[:, hs],
                                        axis=AX.X, op=ALU.add)
                xscr = scr_pool.tile([128, 512], F32,
                                     name=f"xscr{ph}_{k}_{h}", tag="scr")
                nc.scalar.activation(out=xscr, in_=xh[:, hs],
                                     func=AF.Square,
                                     accum_out=o_stats[:, col + 16:col + 17])
                # loss: sum(x-hat * p-hat) over own shard rows
                lscr = scr_pool.tile([128, 512], F32,
                                     name=f"lscr{ph}_{k}_{h}", tag="scr")
                nc.vector.tensor_mul(lscr, xh[:, hs], P[ph][k][h])
                lcol = ph * 8 + k * 2 + h
                nc.vector.tensor_reduce(o_loss[:, lcol:lcol + 1], lscr,
                                        axis=AX.X, op=ALU.add)

        def mm_strip(ph, mb, nt):
            ps = mm_psum.tile([128, NTILE], F32, name=f"ps{ph}_{mb}_{nt}",
                              tag="mm")
            for k in range(KC):
                nc.tensor.matmul(ps, xT[ph][k][:, mb * 128:(mb + 1) * 128],
                                 P[ph][k][nt], start=(k == 0),
                                 stop=(k == KC - 1))
            return ps

        def d_pass(ph):
            # for each strip, compute its diagonal-containing tile first,
            # extract d (bit-exact: identity-mask multiply + reduce), and
            # count that tile on the DVE/is_gt path (self-comparison = 0)
            for mb in range(MB):
                nt_d = (mb * 128) // NTILE
                ps = mm_strip(ph, mb, nt_d)
                sub = (mb * 128) % NTILE
                dscr = scr_pool.tile([128, 128], F32, name=f"dscr{ph}_{mb}",
                                     tag="dscr")
                nc.vector.tensor_mul(dscr, ps[:, sub:sub + 128], i_t)
                nc.vector.tensor_reduce(d_sb[ph][:, mb:mb + 1], dscr,
                                        axis=AX.X, op=ALU.add)
                nc.vector.tensor_scalar_mul(negd_sb[ph][:, mb:mb + 1],
                                            d_sb[ph][:, mb:mb + 1], -1.0)
                cscr = scr_pool.tile([128, NTILE], F32, name=f"cscr{ph}_{mb}",
                                     tag="cscr")
                nc.vector.tensor_scalar(
                    out=cscr, in0=ps, scalar1=d_sb[ph][:, mb:mb + 1],
                    scalar2=0.0, op0=ALU.is_gt, op1=ALU.add,
                    accum_out=cnts[ph][:, mb, nt_d:nt_d + 1])

        def main_col(ph, nt):
            for mb in range(MB):
                if nt == (mb * 128) // NTILE:
                    continue  # handled in the d-pass
                ps = mm_strip(ph, mb, nt)
                if nt in ACT_SET:
                    ascr = scr_pool.tile([128, NTILE], F32,
                                         name=f"a{ph}_{nt}_{mb}", tag="ascr")
                    nc.scalar.activation(
                        out=ascr, in_=ps, func=AF.Sign,
                        bias=negd_sb[ph][:, mb:mb + 1], scale=1.0,
                        accum_out=sgns[ph][:, mb, nt:nt + 1])
                else:
                    cscr = scr_pool.tile([128, NTILE], F32,
                                         name=f"c{ph}_{nt}_{mb}", tag="cscr")
                    nc.vector.tensor_scalar(
                        out=cscr, in0=ps, scalar1=d_sb[ph][:, mb:mb + 1],
                        scalar2=0.0, op0=ALU.is_gt, op1=ALU.add,
                        accum_out=cnts[ph][:, mb, nt:nt + 1])

        def reduce_slots(ph):
            for mb in range(MB):
                c = ph * MB + mb
                nc.vector.tensor_reduce(o_cnt[:, c:c + 1], cnts[ph][:, mb, :],
                                        axis=AX.X, op=ALU.add)
                nc.vector.tensor_reduce(o_sgn[:, c:c + 1], sgns[ph][:, mb, :],
                                        axis=AX.X, op=ALU.add)

        def finalize():
            # per-core partials, packed: [0:16]=stat sums, [16:32]=stat
            # sumsqs (pstat*4+k), [32:40]=retrieval (ph*4+{r1,r5,r10,psum}),
            # [40:42]=loss (ph)
            o_cc = persist.tile([128, 42], F32, name="o_cc")
            # fold the two h-halves of the stats partials: [128,64] -> [128,32]
            nc.vector.tensor_add(o_cc[:, 0:32], o_stats[:, 0:32],
                                 o_stats[:, 32:64])
            # retrieval: pos = cnt + (sgn + C_ACT)/2 per row, then threshold
            # counts + pos-sum per partition
            for ph in range(2):
                sl = slice(ph * MB, (ph + 1) * MB)
                pos = scr_pool.tile([128, MB], F32, name=f"pos{ph}", tag="pos")
                nc.vector.tensor_scalar(out=pos, in0=o_sgn[:, sl],
                                        scalar1=float(C_ACT), scalar2=0.5,
                                        op0=ALU.add, op1=ALU.mult)
                nc.vector.tensor_add(pos, pos, o_cnt[:, sl])
                for j, thr in enumerate((1.0, 5.0, 10.0)):
                    tscr = scr_pool.tile([128, MB], F32,
                                         name=f"tscr{ph}_{j}", tag="tscr")
                    c = 32 + ph * 4 + j
                    nc.vector.tensor_scalar(
                        out=tscr, in0=pos, scalar1=thr, scalar2=0.0,
                        op0=ALU.is_lt, op1=ALU.add,
                        accum_out=o_cc[:, c:c + 1])
                nc.vector.tensor_reduce(o_cc[:, 35 + ph * 4:36 + ph * 4], pos,
                                        axis=AX.X, op=ALU.add)
                nc.vector.tensor_reduce(o_cc[:, 40 + ph:41 + ph],
                                        o_loss[:, ph * 8:(ph + 1) * 8],
                                        axis=AX.X, op=ALU.add)

            # ---- cross-core AllReduce of the partials (16.8KB, on-device) ----
            cc_in = dram_pool.tile([128, 42], F32, name="cc_in")
            cc_out = dram_pool.tile([128, 42], F32, name="cc_out")
            nc.gpsimd.dma_start(cc_in, o_cc)
            nc.gpsimd.collective_compute(
                "AllReduce", ALU.add,
                replica_groups=[list(range(NCORES))],
                ins=[cc_in.opt()], outs=[cc_out.opt()])
            g = persist.tile([128, 42], F32, name="g")
            nc.gpsimd.dma_start(g, cc_out)

            # ---- final math on device (identical on every core) ----
            # std per feature: var = (ssq - s*s/N)/(N-1), clamped, sqrt
            h_t = persist.tile([128, 26], F32, name="h_t")
            var = scr_pool.tile([128, 16], F32, name="var", tag="var")
            nc.vector.tensor_mul(var, g[:, 0:16], g[:, 0:16])
            nc.vector.tensor_scalar_mul(var, var, -1.0 / N)
            nc.vector.tensor_add(var, var, g[:, 16:32])
            nc.vector.tensor_scalar_mul(var, var, 1.0 / (N - 1))
            nc.vector.tensor_scalar_max(var, var, 0.0)
            nc.scalar.sqrt(h_t[:, 0:16], var)
            nc.vector.tensor_copy(h_t[:, 16:26], g[:, 32:42])
            # cross-partition sums via ones-vector matmul: [1,26]
            ones1 = persist.tile([128, 1], F32, name="ones1")
            nc.vector.memset(ones1, 1.0)
            ps_t = nrm_psum.tile([128, 26], F32, name="ps_fin", tag="nrm")
            ps_f = ps_t[0:1, :]
            nc.tensor.matmul(ps_f, ones1, h_t, start=True, stop=True)
            # assemble the 13-vector: [neg_sim, stats v,t,pv,pt, v_r1,v_r5,
            # v_r10,v_mr, t_r1,t_r5,t_r10,t_mr, 0,0,0]
            o_fin = persist.tile([1, 16], F32, name="o_fin")
            nc.vector.memset(o_fin, 0.0)
            for t in range(4):  # mean std over the 512 feature columns
                nc.vector.tensor_reduce(o_fin[:, 1 + t:2 + t],
                                        ps_f[:, t * 4:(t + 1) * 4],
                                        axis=AX.X, op=ALU.add)
            nc.vector.tensor_scalar_mul(o_fin[:, 1:5], o_fin[:, 1:5], 1.0 / C)
            nc.vector.tensor_scalar_mul(o_fin[:, 5:13], ps_f[:, 16:24], 1.0 / N)
            lsum = scr_pool.tile([1, 1], F32, name="lsum", tag="lsum")
            nc.vector.tensor_reduce(lsum, ps_f[:, 24:26], axis=AX.X, op=ALU.add)
            nc.vector.tensor_scalar_mul(o_fin[:, 0:1], lsum, -0.5 / N)
            nc.sync.dma_start(out=o_all_d, in_=o_fin)

        # ---- emission order (Tile priority / engine-FIFO order follows
        # program order, so interleave cross-phase work deliberately):
        # the P-column load+normalize stream leads the matmul+count stream by
        # two columns, and phase 1's loads trail phase 0's last reader. ----
        load_and_norm_P(0, 0)
        for k in range(KC):
            nc.sync.dma_start(out=xT[0][k],
                              in_=xT_d[0][k * 128:(k + 1) * 128, :])
        load_and_norm_P(0, 1)
        x_norm(0)
        d_pass(0)
        for nt in range(2, NT):
            load_and_norm_P(0, nt)
            m = nt - 2
            main_col(0, m)
            if m < KC:
                stats_chunk(0, m)
            if m == KC:
                for k in range(KC):
                    nc.sync.dma_start(out=xT[1][k],
                                      in_=xT_d[1][k * 128:(k + 1) * 128, :])
                x_norm(1)
            if m >= 5:
                load_and_norm_P(1, m - 5)
        main_col(0, NT - 2)
        load_and_norm_P(1, 9)
        main_col(0, NT - 1)
        load_and_norm_P(1, 10)
        for j in range(11, NT):
            load_and_norm_P(1, j)
        reduce_slots(0)
        d_pass(1)
        for nt in range(NT):
            main_col(1, nt)
            if nt < KC:
                stats_chunk(1, nt)
        reduce_slots(1)
        finalize()

    nc.compile()
    return nc


def _get_runner():
    """Build (once) a jitted 8-core SPMD executor for the Bass program.

    Mirrors bass2jax.run_bass_via_pjrt's multi-core branch, but keeps the
    jitted function and pre-staged device inputs so repeated calls skip
    retracing/recompiling, and so transfer vs execute can be timed apart.
    """
    if "runner" in _CACHE:
        return _CACHE["runner"]

    import jax
    import jax.numpy as jnp
    from jax.experimental.shard_map import shard_map
    from jax.sharding import Mesh, PartitionSpec, NamedSharding
    from concourse import mybir as _mybir
    from concourse.bass2jax import (_bass_exec_p, install_neuronx_cc_hook,
                                    partition_id_tensor)

    nc = _CACHE["nc"]
    install_neuronx_cc_hook()

    partition_name = (nc.partition_id_tensor.name
                      if nc.partition_id_tensor else None)
    in_names, out_names, out_avals = [], [], []
    zero_outs = []
    for alloc in nc.m.functions[0].allocations:
        if not isinstance(alloc, _mybir.MemoryLocationSet):
            continue
        name = alloc.memorylocations[0].name
        if alloc.kind == "ExternalInput":
            if name != partition_name:
                in_names.append(name)
        elif alloc.kind == "ExternalOutput":
            out_names.append(name)
            shape = tuple(alloc.tensor_shape)
            dtype = _mybir.dt.np(alloc.dtype)
            out_avals.append(jax.core.ShapedArray(shape, dtype))
            zero_outs.append(np.zeros(shape, dtype))
    n_params = len(in_names)
    all_in_names = in_names + out_names
    if partition_name is not None:
        all_in_names = all_in_names + [partition_name]

    def _body(*args):
        operands = list(args)
        if partition_name is not None:
            operands.append(partition_id_tensor())
        outs = _bass_exec_p.bind(
            *operands,
            out_avals=tuple(out_avals),
            in_names=tuple(all_in_names),
            out_names=tuple(out_names),
            lowering_input_output_aliases=(),
            sim_require_finite=True,
            sim_require_nnan=True,
            nc=nc,
        )
        return tuple(outs)

    devices = jax.devices()[:NCORES]
    mesh = Mesh(np.asarray(devices), ("core",))
    spec = NamedSharding(mesh, PartitionSpec("core"))
    donate = tuple(range(n_params, n_params + len(out_names)))
    sharded = jax.jit(
        shard_map(_body, mesh=mesh,
                  in_specs=(PartitionSpec("core"),) * (n_params + len(out_names)),
                  out_specs=(PartitionSpec("core"),) * len(out_names),
                  check_rep=False),
        donate_argnums=donate, keep_unused=True)

    # AOT-lower now (inside the untimed build phase) so the first executed
    # call doesn't pay jit tracing/lowering in its timed region
    in_shapes = {}
    for alloc in nc.m.functions[0].allocations:
        if isinstance(alloc, _mybir.MemoryLocationSet):
            in_shapes[alloc.memorylocations[0].name] = (
                tuple(alloc.tensor_shape), _mybir.dt.np(alloc.dtype))
    arg_structs = [
        jax.ShapeDtypeStruct(
            (NCORES * in_shapes[nm][0][0],) + in_shapes[nm][0][1:],
            in_shapes[nm][1], sharding=spec)
        for nm in in_names + out_names
    ]
    sharded = sharded.lower(*arg_structs).compile()

    def run(in_maps, _retries=2):
        t0 = time.time()
        concat_in = [
            np.concatenate([in_maps[c][name] for c in range(NCORES)], axis=0)
            for name in in_names
        ]
        try:
            return _run_staged(concat_in, t0)
        except Exception:
            # devices occasionally wedge (NRT_EXEC_UNIT_UNRECOVERABLE);
            # a fresh attempt sometimes recovers within the process
            if _retries <= 0:
                raise
            time.sleep(2.0)
            return run(in_maps, _retries - 1)

    def _run_staged(concat_in, t0):
        dev_in = [jax.device_put(a, spec) for a in concat_in]
        dev_zero = [jax.device_put(
            np.zeros((NCORES * z.shape[0], *z.shape[1:]), z.dtype), spec)
            for z in zero_outs]
        for a in dev_in + dev_zero:
            a.block_until_ready()
        t1 = time.time()
        out_arrs = sharded(*dev_in, *dev_zero)
        td = time.time()
        if FETCH_SHARD0:
            # every core returns the identical reduced vector, so fetch only
            # device 0's shard (one d2h RPC instead of eight overlapped ones)
            shard0 = [np.asarray(a.addressable_shards[0].data)
                      for a in out_arrs]
            t2 = time.time()
            TIMES.update(transfer_s=t1 - t0, execute_s=t2 - t1,
                         dispatch_s=td - t1, fetch_s=t2 - td)
            return [
                {name: shard0[i] for i, name in enumerate(out_names)}
                for c in range(NCORES)
            ]
        # one batched fetch: d2h rounds through the axon tunnel cost ~81ms
        # each but overlap perfectly, so fetch everything in one call
        out_np = jax.device_get(list(out_arrs))
        t2 = time.time()
        TIMES.update(transfer_s=t1 - t0, execute_s=t2 - t1,
                     dispatch_s=td - t1, fetch_s=t2 - td)
        return [
            {name: out_np[i].reshape(NCORES, *out_avals[i].shape)[c]
             for i, name in enumerate(out_names)}
            for c in range(NCORES)
        ]

    _CACHE["runner"] = run
    return run


TIMES = {}
FETCH_SHARD0 = False


def kernel(v_feat, t_feat, p_v, p_t):
    if "nc" not in _CACHE:
        _CACHE["nc"] = _build_program()

    t0 = time.time()
    v = np.ascontiguousarray(v_feat, dtype=np.float32)
    t = np.ascontiguousarray(t_feat, dtype=np.float32)
    pv = np.ascontiguousarray(p_v, dtype=np.float32)
    pt = np.ascontiguousarray(p_t, dtype=np.float32)

    ident = np.eye(128, dtype=np.float32)
    ones = np.ones((128, 128), dtype=np.float32)

    in_maps = []
    for k in range(NCORES):
        sl = slice(k * S, (k + 1) * S)
        in_maps.append({
            "vT": np.ascontiguousarray(v[sl].T),
            "tT": np.ascontiguousarray(t[sl].T),
            "ptT": np.ascontiguousarray(np.roll(pt, -k * S, axis=0).T),
            "pvT": np.ascontiguousarray(np.roll(pv, -k * S, axis=0).T),
            "ident": ident,
            "ones": ones,
        })
    TIMES["prep_s"] = time.time() - t0

    results = _get_runner()(in_maps)

    # the whole reduction ran on device (AllReduce across cores + final
    # math); every core returns the identical padded 13-vector
    out = np.asarray(results[0]["o_all"][0][:13], dtype=np.float32)
    return out

